# revision 1
# baseline (speedup 1.0000x reference)
"""Trainium2 Bass kernel for nn_BasicTransformerBlock (self-contained).


Sharding: sequence-parallel. 8 cores = 2 batch groups x 4 cores.
Each core owns TOK=512 tokens of one batch element. Attention needs the
full 2048-token context of that batch element, obtained by AllGathering
K^T and V(+ones column) within the 4-core group.

Layout conventions per core:
  x residual stream: token-major [512, 1024] fp32, as 4 tiles [128, 1024].
  xlnT: LayerNorm'd x, transposed: [1024, 512] bf16 as 8 tiles [128, 512].
  Q^T, K^T: feature-major [1024, tok] bf16.
  V: token-major [tok, 1040] bf16 with a ones column appended per head
     (column h*65+64), so the AV matmul also produces the softmax
     denominator (row 64 of the [65, tok] output).
  Scores S^T: [j, q] per head in [128, 1024] psum tiles (2 j-tiles each);
     one Exp per psum tile -> P^T bf16 in SBUF.
  attn out^T: feature-major [1024, tok] bf16 after per-head 1/denom scale.
"""


from contextlib import ExitStack

import numpy as np

import concourse.bass as bass
import concourse.mybir as mybir
import concourse.tile as tile
from concourse.tile_rust import add_dep_helper
from concourse import bacc
from concourse.masks import make_identity

F32 = mybir.dt.float32
BF16 = mybir.dt.bfloat16
AX = mybir.AxisListType.X
AF = mybir.ActivationFunctionType
ALU = mybir.AluOpType

D = 1024          # model dim
HEADS = 16
DH = 64
FF = 4096         # ff inner (per half)
EPS = 1e-5
P = 128


class Ctx:
    """Holds the bass handles shared across emit stages."""


def build(group: int, tok: int, use_bias: bool = False):
    """group: cores per batch group (1 = no collective, 4 = real).
    tok: local tokens per core (512)."""
    ntok = group * tok
    TT = tok // P          # local token tiles (4)
    JT = ntok // P         # context token tiles (16 when group=4)

    nc = bacc.Bacc("TRN2", target_bir_lowering=False, debug=False,
                   num_devices=8)

    c = Ctx()
    c.nc = nc
    c.group = group
    c.tok = tok
    c.ntok = ntok
    c.TT = TT
    c.JT = JT
    c.use_bias = use_bias

    # ---- I/O ----
    c.x_in = nc.dram_tensor("x", [tok, D], F32, kind="ExternalInput")
    c.y_out = nc.dram_tensor("y", [tok, D], F32, kind="ExternalOutput")
    w = {}
    for i in (1, 2):
        w[f"wq{i}"] = nc.dram_tensor(f"wq{i}", [D, D], BF16, kind="ExternalInput")
        w[f"wk{i}"] = nc.dram_tensor(f"wk{i}", [D, D], BF16, kind="ExternalInput")
        w[f"wv{i}"] = nc.dram_tensor(f"wv{i}", [D, D], BF16, kind="ExternalInput")
        w[f"wo{i}"] = nc.dram_tensor(f"wo{i}", [D, D], BF16, kind="ExternalInput")
    w["wf1"] = nc.dram_tensor("wf1", [32, P, 2048], BF16, kind="ExternalInput")
    w["wf2"] = nc.dram_tensor("wf2", [FF, D], BF16, kind="ExternalInput")
    if use_bias:
        # per-projection bias vectors (LN beta pushed through W, plus the
        # projection's own bias where it exists)
        for i in (1, 2):
            for nmv in ("cq", "ck", "cv", "bo"):
                w[f"{nmv}{i}"] = nc.dram_tensor(f"{nmv}{i}", [1, D], BF16,
                                                kind="ExternalInput")
        w["c1"] = nc.dram_tensor("c1", [1, 2 * FF], BF16, kind="ExternalInput")
        w["bf2"] = nc.dram_tensor("bf2", [1, D], BF16, kind="ExternalInput")
    c.w = w

    with ExitStack() as stack:
        tc = stack.enter_context(tile.TileContext(nc))
        c.tc = tc

        const = stack.enter_context(tc.tile_pool(name="const", bufs=1))
        c.identity = const.tile([P, P], BF16, name="identity")
        make_identity(nc, c.identity)
        c.ones_f32 = const.tile([1, 64], F32, name="ones_f32")
        nc.vector.memset(c.ones_f32, 1.0)
        c.eps_tile = const.tile([P, 1], F32, name="eps_tile")
        nc.vector.memset(c.eps_tile, EPS)
        if use_bias:
            c.ones_bf = const.tile([1, tok], BF16, name="ones_bf")
            nc.vector.memset(c.ones_bf, 1.0)
            c.bias_sb = {}
            for key, t in w.items():
                if key[0] in "cb" and key not in ("c1",):
                    bt = const.tile([1, D], BF16, name=f"sb_{key}")
                    nc.sync.dma_start(bt, t[:, :])
                    c.bias_sb[key] = bt
            bt = const.tile([1, 2 * FF], BF16, name="sb_c1")
            nc.sync.dma_start(bt, w["c1"][:, :])
            c.bias_sb["c1"] = bt

        xres_pool = stack.enter_context(tc.tile_pool(name="xres", bufs=1))
        c.xres = [xres_pool.tile([P, D], F32, name=f"xres{t}") for t in range(TT)]
        for t in range(TT):
            nc.sync.dma_start(c.xres[t], c.x_in[t * P:(t + 1) * P, :])

        # DRAM bounce buffers: half-sized K/V allgathers (per attention)
        if group > 1:
            ke = D * tok               # whole K^T
            vh = tok * (HEADS // 2) * 65   # half of V(+ones)
            dram = stack.enter_context(
                tc.tile_pool(name="dram", bufs=1, space="DRAM"))
            c.k_in = [dram.tile([ke], BF16, name=f"k_in{i}")
                      for i in (0, 1)]
            c.k_out = [dram.tile([group * ke], BF16, name=f"k_out{i}")
                       for i in (0, 1)]
            c.v_in = [[dram.tile([vh], BF16, name=f"v_in{i}_{half}")
                       for half in (0, 1)] for i in (0, 1)]
            c.v_out = [[dram.tile([group * vh], BF16, name=f"v_out{i}_{half}")
                        for half in (0, 1)] for i in (0, 1)]

        emit_attn(c, 1)
        emit_attn(c, 2)
        emit_ff(c)

        for t in range(TT):
            nc.sync.dma_start(c.y_out[t * P:(t + 1) * P, :], c.xres[t])

    nc.compile()
    return nc


def emit_ln_transpose(c, outer, name):
    """LayerNorm xres (gain/bias pre-folded into weights) and produce
    xlnT: 8 tiles [128, tok] bf16 (transposed normalized x).
    xlnT tiles live in `outer`; scratch pools are closed on return."""
    nc, tc = c.nc, c.tc
    TT = c.TT

    xlnT_pool = outer.enter_context(tc.tile_pool(name=f"{name}_xlnT", bufs=1))
    xlnT = [xlnT_pool.tile([P, c.tok], BF16, name=f"{name}_xlnT{d}")
            for d in range(8)]

    with ExitStack() as ph:
        pool = ph.enter_context(tc.tile_pool(name=f"{name}_ln", bufs=2))
        psum_sq = ph.enter_context(
            tc.tile_pool(name=f"{name}_psq", bufs=2, space="PSUM"))
        psum_tr = ph.enter_context(
            tc.tile_pool(name=f"{name}_ptr", bufs=4, space="PSUM"))

        for t in range(TT):
            xr = c.xres[t]
            # var = E[x^2] - mu^2: sum and sumsq run concurrently (DVE / ACT)
            ssum = pool.tile([P, 1], F32, tag="ssum", name=f"{name}_ssum{t}")
            nc.vector.reduce_sum(ssum, xr, axis=AX)
            sq_sink = psum_sq.tile([P, D], F32, tag="sq", name=f"{name}_sq{t}")
            sumsq = pool.tile([P, 1], F32, tag="sumsq", name=f"{name}_vs{t}")
            nc.scalar.activation(sq_sink, xr, AF.Square, accum_out=sumsq)
            mu = pool.tile([P, 1], F32, tag="mu", name=f"{name}_mu{t}")
            nc.vector.tensor_scalar_mul(mu, ssum, 1.0 / D)
            musq = pool.tile([P, 1], F32, tag="musq", name=f"{name}_msq{t}")
            nc.vector.tensor_mul(musq, mu, mu)
            bvar = pool.tile([P, 1], F32, tag="bvar", name=f"{name}_bv{t}")
            nc.vector.tensor_scalar(bvar, musq, -1.0, EPS,
                                    op0=ALU.mult, op1=ALU.add)
            std = pool.tile([P, 1], F32, tag="std", name=f"{name}_std{t}")
            nc.scalar.activation(std, sumsq, AF.Sqrt, bias=bvar,
                                 scale=1.0 / D)
            rstd = pool.tile([P, 1], F32, tag="rstd", name=f"{name}_rstd{t}")
            nc.vector.reciprocal(rstd, std)
            nmr = pool.tile([P, 1], F32, tag="nmr", name=f"{name}_nmr{t}")
            nc.vector.tensor_mul(nmr, mu, rstd)
            nc.vector.tensor_scalar_mul(nmr, nmr, -1.0)
            xln = pool.tile([P, D], BF16, tag="xln", name=f"{name}_xln{t}")
            nc.vector.tensor_scalar(xln, xr, rstd, nmr,
                                    op0=ALU.mult, op1=ALU.add)
            for dc in range(8):
                tp = psum_tr.tile([P, P], BF16, tag="tp",
                                  name=f"{name}_tp{t}_{dc}")
                nc.tensor.transpose(tp, xln[:, dc * P:(dc + 1) * P],
                                    c.identity)
                nc.vector.tensor_copy(xlnT[dc][:, t * P:(t + 1) * P], tp)
    return xlnT


def proj_fm(c, ph, name, xlnT, w_dram, out_tiles, bias_key=None):
    """Feature-major projection: out^T[m] [128, tok] = W'-chunk.T @ xlnT.
    Streams weights and psum within this scope."""
    nc, tc = c.nc, c.tc
    with ExitStack() as sub:
        wpool = sub.enter_context(tc.tile_pool(name=f"{name}_w", bufs=1))
        w_tiles = []
        for kd in range(8):
            wt = wpool.tile([P, D], BF16, name=f"{name}_w{kd}")
            nc.sync.dma_start(wt, w_dram[kd * P:(kd + 1) * P, :])
            w_tiles.append(wt)
        psum = sub.enter_context(
            tc.tile_pool(name=f"{name}_ps", bufs=3, space="PSUM"))
        has_bias = c.use_bias and bias_key is not None
        for m in range(8):
            ps = psum.tile([P, c.tok], F32, tag="proj", name=f"{name}_ps{m}")
            for kd in range(8):
                nc.tensor.matmul(ps, lhsT=w_tiles[kd][:, m * P:(m + 1) * P],
                                 rhs=xlnT[kd], start=(kd == 0),
                                 stop=(kd == 7 and not has_bias))
            if has_bias:
                nc.tensor.matmul(
                    ps, lhsT=c.bias_sb[bias_key][0:1, m * P:(m + 1) * P],
                    rhs=c.ones_bf, start=False, stop=True)
            nc.vector.tensor_copy(out_tiles[m], ps)


def emit_attn(c, idx):
    nc, tc = c.nc, c.tc
    name = f"a{idx}"
    TT, JT, tok = c.TT, c.JT, c.tok
    RG = [[0, 1, 2, 3], [4, 5, 6, 7]]
    vw = HEADS * 65
    vhw = vw // 2

    with ExitStack() as ph:
        xlnT = emit_ln_transpose(c, ph, name)

        kfull_pool = ph.enter_context(tc.tile_pool(name=f"{name}_kf", bufs=1))
        kT_m = [kfull_pool.tile([P, c.ntok], BF16, name=f"{name}_kTm{m}")
                for m in range(8)]
        vones_hr = [[kfull_pool.tile([P, TT, vhw], BF16,
                                     name=f"{name}_vo{h}_{r}")
                     for r in range(c.group if c.group > 1 else 1)]
                    for h in (0, 1)]
        qT = [kfull_pool.tile([P, tok], BF16, name=f"{name}_qT{m}")
              for m in range(8)]

        stage_stack = ExitStack()
        kv_pool = stage_stack.enter_context(
            tc.tile_pool(name=f"{name}_kv", bufs=1))
        kT_loc = [kv_pool.tile([P, tok], BF16, name=f"{name}_kTl{m}")
                  for m in range(8)]
        vstage = [kv_pool.tile([P, vw], BF16, name=f"{name}_vst{t}")
                  for t in range(TT)]

        # --- K^T projection; kick the K allgather when it completes ---
        proj_fm(c, ph, f"{name}_kproj", xlnT, c.w[f"wk{idx}"], kT_loc,
                bias_key=f"ck{idx}")
        if c.group > 1:
            k_in = c.k_in[idx - 1]
            for m in range(8):
                nc.sync.dma_start(
                    k_in[m * P * tok:(m + 1) * P * tok]
                    .rearrange("(p f) -> p f", f=tok), kT_loc[m])
            nc.gpsimd.collective_compute(
                "AllGather", ALU.bypass, replica_groups=RG,
                ins=[c.k_in[idx - 1][:]], outs=[c.k_out[idx - 1][:]])

        # --- V(+ones) projection, n-major so head-half 0 finishes first ---
        with ExitStack() as sub:
            wpool = sub.enter_context(tc.tile_pool(name=f"{name}_wvp", bufs=1))
            wv_tiles = []
            for kd in range(8):
                wt = wpool.tile([P, D], BF16, name=f"{name}_wv{kd}")
                nc.sync.dma_start(wt, c.w[f"wv{idx}"][kd * P:(kd + 1) * P, :])
                wv_tiles.append(wt)
            psum = sub.enter_context(
                tc.tile_pool(name=f"{name}_vps", bufs=2, space="PSUM"))
            for t in range(TT):
                nc.vector.memset(
                    vstage[t].rearrange("p (h e) -> p h e", e=65)[:, :, 64:65],
                    1.0)
            # t-outer so the stationary xlnT slice is loaded once per
            # (t, kd) and reused for both n-chunks (halves the LDWEIGHTS)
            pss = {}
            for t in range(TT):
                for n in range(2):
                    pss[(t, n)] = psum.tile([P, 512], F32, tag=f"vp{n}",
                                            name=f"{name}_vps{t}_{n}")
                for kd in range(8):
                    for n in range(2):
                        nc.tensor.matmul(
                            pss[(t, n)],
                            lhsT=xlnT[kd][:, t * P:(t + 1) * P],
                            rhs=wv_tiles[kd][:, n * 512:(n + 1) * 512],
                            start=(kd == 0),
                            stop=(kd == 7 and not c.use_bias))
                for n in range(2):
                    if c.use_bias:
                        nc.tensor.matmul(
                            pss[(t, n)], lhsT=c.ones_bf[0:1, 0:P],
                            rhs=c.bias_sb[f"cv{idx}"][0:1,
                                                      n * 512:(n + 1) * 512],
                            start=False, stop=True)
                    dst = vstage[t][:, n * 520:(n + 1) * 520].rearrange(
                        "p (h e) -> p h e", e=65)[:, :, 0:64]
                    nc.vector.tensor_copy(
                        dst, pss[(t, n)].rearrange("p (h e) -> p h e", e=64))
            if c.group > 1:
                for n in range(2):
                    v_in = c.v_in[idx - 1][n]
                    for t in range(TT):
                        nc.sync.dma_start(
                            v_in[t * P * vhw:(t + 1) * P * vhw]
                            .rearrange("(p f) -> p f", f=vhw),
                            vstage[t][:, n * 520:(n + 1) * 520])
                    if n == 0:
                        nc.gpsimd.collective_compute(
                            "AllGather", ALU.bypass, replica_groups=RG,
                            ins=[v_in[:]], outs=[c.v_out[idx - 1][0][:]])

        # --- Q^T projection (overlaps the gathers) ---
        proj_fm(c, ph, f"{name}_qproj", xlnT, c.w[f"wq{idx}"], qT,
                bias_key=f"cq{idx}")

        if c.group == 1:
            for m in range(8):
                nc.vector.tensor_copy(kT_m[m], kT_loc[m])
            for t in range(TT):
                for half in (0, 1):
                    nc.scalar.copy(vones_hr[half][0][:, t, :],
                                   vstage[t][:, half * 520:(half + 1) * 520])
            stage_stack.close()
        else:
            # remaining V half-gather, then read everything back.
            nc.gpsimd.collective_compute(
                "AllGather", ALU.bypass, replica_groups=RG,
                ins=[c.v_in[idx - 1][1][:]], outs=[c.v_out[idx - 1][1][:]])
            stage_stack.close()
            def read_k(half):
                k_out = c.k_out[idx - 1]
                for mm in range(4):
                    for r in range(c.group):
                        m = half * 4 + mm
                        nc.sync.dma_start(
                            kT_m[m][:, r * tok:(r + 1) * tok],
                            k_out[(r * 8 + m) * P * tok:
                                  (r * 8 + m + 1) * P * tok]
                            .rearrange("(p f) -> p f", f=tok))

            def read_v(half):
                v_out = c.v_out[idx - 1][half]
                for r in range(c.group):
                    for t in range(TT):
                        nc.sync.dma_start(
                            vones_hr[half][r][:, t, :],
                            v_out[(r * TT + t) * P * vhw:
                                  (r * TT + t + 1) * P * vhw]
                            .rearrange("(p f) -> p f", f=vhw))

            read_k(0)
            read_k(1)
            read_v(0)
            read_v(1)

        # prefetch out-projection weights while heads run
        wo_pool = ph.enter_context(tc.tile_pool(name=f"{name}_wop", bufs=1))
        wo_tiles = []
        for m in range(8):
            wt = wo_pool.tile([P, D], BF16, name=f"{name}_wo{m}")
            nc.sync.dma_start(wt, c.w[f"wo{idx}"][m * P:(m + 1) * P, :])
            wo_tiles.append(wt)

        # --- attention, head pairs, software-pipelined ---
        attnT_pool = ph.enter_context(tc.tile_pool(name=f"{name}_at", bufs=1))
        attnT = [attnT_pool.tile([P, tok], BF16, name=f"{name}_attnT{m}")
                 for m in range(8)]
        NR = JT // 2           # score rounds per head (2 j-tiles per round)

        with ExitStack() as sub:
            psum_sc = sub.enter_context(
                tc.tile_pool(name=f"{name}_psc", bufs=2, space="PSUM"))
            psum_av = sub.enter_context(
                tc.tile_pool(name=f"{name}_pav", bufs=3, space="PSUM"))
            psum_bc = sub.enter_context(
                tc.tile_pool(name=f"{name}_pbc", bufs=1, space="PSUM"))
            pT_pool = sub.enter_context(
                tc.tile_pool(name=f"{name}_pT", bufs=22))
            small = sub.enter_context(
                tc.tile_pool(name=f"{name}_small", bufs=2))

            pending_epilogue = None

            def emit_epilogue(ep):
                av_pair, m2 = ep
                for s in range(2):
                    po2 = s * 64
                    den1 = small.tile([1, tok], F32, tag="den",
                                      name=f"{name}_den{m2}_{s}")
                    nc.vector.tensor_copy(den1, av_pair[s][64:65, :])
                    rden = small.tile([1, tok], F32, tag="rden",
                                      name=f"{name}_rd{m2}_{s}")
                    nc.vector.reciprocal_approx_fast(rden, den1)
                    ps_b = psum_bc.tile([64, tok], F32, tag="bc",
                                        name=f"{name}_bc{m2}_{s}")
                    nc.tensor.matmul(ps_b, lhsT=c.ones_f32[0:1, 0:64],
                                     rhs=rden, start=True, stop=True)
                    rbc = small.tile([64, tok], F32, tag="rbc",
                                     name=f"{name}_rbc{m2}_{s}")
                    nc.vector.tensor_copy(rbc, ps_b)
                    nc.vector.tensor_tensor(attnT[m2][po2:po2 + 64, :],
                                            av_pair[s][0:64, :], rbc,
                                            op=ALU.mult)

            for m in range(8):       # head pair (2m, 2m+1)
                av_pair = [psum_av.tile([P, tok], F32, tag="av",
                                        name=f"{name}_av{m}_{s}")
                           for s in range(2)]
                pend = None          # (pA_pB, r)
                for r in range(NR):
                    ps2 = [psum_sc.tile([P, 2, tok], F32, tag="sc",
                                        name=f"{name}_sc{m}_{r}_{s}")
                           for s in range(2)]
                    for u in range(2):
                        jt = 2 * r + u
                        for s in range(2):
                            po = s * 64
                            nc.tensor.matmul(
                                ps2[s][:, u, :],
                                lhsT=kT_m[m][po:po + 64,
                                             jt * P:(jt + 1) * P],
                                rhs=qT[m][po:po + 64, :],
                                start=True, stop=True)
                    if r == 1 and pending_epilogue is not None:
                        emit_epilogue(pending_epilogue)
                        pending_epilogue = None
                    p2 = []
                    for s in range(2):
                        p_sb = pT_pool.tile([P, 2, tok], BF16, tag="pT",
                                            name=f"{name}_p{m}_{r}_{s}")
                        nc.scalar.activation(p_sb, ps2[s], AF.Exp)
                        p2.append(p_sb)
                    if pend is not None:
                        pp, rr = pend
                        for u in range(2):
                            jt = 2 * rr + u
                            for s in range(2):
                                h = 2 * m + s
                                nc.tensor.matmul(
                                    av_pair[s][0:65, :],
                                    lhsT=vones_hr[h // 8][
                                        jt // TT if c.group > 1 else 0][
                                        :, jt % TT,
                                        (h % 8) * 65:(h % 8 + 1) * 65],
                                    rhs=pp[s][:, u, :],
                                    start=(jt == 0), stop=(jt == JT - 1))
                    pend = (p2, r)
                pp, rr = pend
                for u in range(2):
                    jt = 2 * rr + u
                    for s in range(2):
                        h = 2 * m + s
                        nc.tensor.matmul(
                            av_pair[s][0:65, :],
                            lhsT=vones_hr[h // 8][
                                jt // TT if c.group > 1 else 0][
                                :, jt % TT,
                                (h % 8) * 65:(h % 8 + 1) * 65],
                            rhs=pp[s][:, u, :],
                            start=(jt == 0), stop=(jt == JT - 1))
                pending_epilogue = (av_pair, m)
            emit_epilogue(pending_epilogue)

        # --- out projection + residual (weights prefetched pre-heads) ---
        with ExitStack() as sub:
            psum_o = sub.enter_context(
                tc.tile_pool(name=f"{name}_po", bufs=1, space="PSUM"))
            ps_o = {}
            for t in range(TT):
                for n in range(2):
                    ps_o[(t, n)] = psum_o.tile([P, 512], F32, tag=f"o{t}_{n}",
                                               name=f"{name}_pso{t}_{n}")
            for m in range(8):
                for t in range(TT):
                    for n in range(2):
                        nc.tensor.matmul(
                            ps_o[(t, n)],
                            lhsT=attnT[m][:, t * P:(t + 1) * P],
                            rhs=wo_tiles[m][:, n * 512:(n + 1) * 512],
                            start=(m == 0),
                            stop=(m == 7 and not c.use_bias))
            if c.use_bias:
                for t in range(TT):
                    for n in range(2):
                        nc.tensor.matmul(
                            ps_o[(t, n)], lhsT=c.ones_bf[0:1, 0:P],
                            rhs=c.bias_sb[f"bo{idx}"][0:1,
                                                      n * 512:(n + 1) * 512],
                            start=False, stop=True)
            for t in range(TT):
                for n in range(2):
                    sl = slice(n * 512, (n + 1) * 512)
                    nc.vector.tensor_add(c.xres[t][:, sl], c.xres[t][:, sl],
                                         ps_o[(t, n)])


def emit_ff(c):
    nc, tc = c.nc, c.tc
    name = "ff"
    TT, tok = c.TT, c.tok

    with ExitStack() as ph:
        xlnT = emit_ln_transpose(c, ph, name)

        h2_pool = ph.enter_context(tc.tile_pool(name=f"{name}_h2", bufs=1))
        h2T = [h2_pool.tile([P, tok], BF16, name=f"{name}_h2T{m}")
               for m in range(32)]

        with ExitStack() as sub:
            f1_pool = sub.enter_context(
                tc.tile_pool(name=f"{name}_f1", bufs=6))
            psum_ff = sub.enter_context(
                tc.tile_pool(name=f"{name}_pff", bufs=3, space="PSUM"))
            gl_pool = sub.enter_context(
                tc.tile_pool(name=f"{name}_gl", bufs=3))

            for pm in range(32):
                f1 = f1_pool.tile([P, 8, 256], BF16, tag="f1",
                                  name=f"{name}_f1_{pm}")
                nc.sync.dma_start(
                    f1.rearrange("p a b -> p (a b)"), c.w["wf1"][pm, :, :])
                ps_a = psum_ff.tile([P, tok], F32, tag="ffa",
                                    name=f"{name}_fa{pm}")
                ps_g = psum_ff.tile([P, tok], F32, tag="ffg",
                                    name=f"{name}_fg{pm}")
                for kd in range(8):
                    nc.tensor.matmul(ps_a, lhsT=f1[:, kd, 0:128],
                                     rhs=xlnT[kd], start=(kd == 0),
                                     stop=(kd == 7 and not c.use_bias))
                for kd in range(8):
                    nc.tensor.matmul(ps_g, lhsT=f1[:, kd, 128:256],
                                     rhs=xlnT[kd], start=(kd == 0),
                                     stop=(kd == 7 and not c.use_bias))
                if c.use_bias:
                    nc.tensor.matmul(
                        ps_a, lhsT=c.bias_sb["c1"][0:1, pm * 256:pm * 256 + 128],
                        rhs=c.ones_bf, start=False, stop=True)
                    nc.tensor.matmul(
                        ps_g,
                        lhsT=c.bias_sb["c1"][0:1, pm * 256 + 128:pm * 256 + 256],
                        rhs=c.ones_bf, start=False, stop=True)
                gl = gl_pool.tile([P, tok], BF16, tag="gelu",
                                  name=f"{name}_gl{pm}")
                nc.scalar.activation(gl, ps_g, AF.Gelu)
                nc.vector.tensor_tensor(h2T[pm], ps_a, gl, op=ALU.mult)

        # FF2 + residual
        with ExitStack() as sub:
            wf2_pool = sub.enter_context(
                tc.tile_pool(name=f"{name}_w2", bufs=8))
            psum_o = sub.enter_context(
                tc.tile_pool(name=f"{name}_po2", bufs=1, space="PSUM"))
            ps_o = {}
            for t in range(TT):
                for n in range(2):
                    ps_o[(t, n)] = psum_o.tile([P, 512], F32, tag=f"o{t}_{n}",
                                               name=f"{name}_pso{t}_{n}")
            for m in range(32):
                w2 = wf2_pool.tile([P, D], BF16, tag="w2",
                                   name=f"{name}_w2_{m}")
                nc.sync.dma_start(w2, c.w["wf2"][m * P:(m + 1) * P, :])
                for t in range(TT):
                    for n in range(2):
                        nc.tensor.matmul(
                            ps_o[(t, n)],
                            lhsT=h2T[m][:, t * P:(t + 1) * P],
                            rhs=w2[:, n * 512:(n + 1) * 512],
                            start=(m == 0),
                            stop=(m == 31 and not c.use_bias))
            if c.use_bias:
                for t in range(TT):
                    for n in range(2):
                        nc.tensor.matmul(
                            ps_o[(t, n)], lhsT=c.ones_bf[0:1, 0:P],
                            rhs=c.bias_sb["bf2"][0:1, n * 512:(n + 1) * 512],
                            start=False, stop=True)
            for t in range(TT):
                for n in range(2):
                    sl = slice(n * 512, (n + 1) * 512)
                    nc.vector.tensor_add(c.xres[t][:, sl], c.xres[t][:, sl],
                                         ps_o[(t, n)])


# ---------------- host-side helpers ----------------

def prep_weights(inp):
    """Fold LN gains + attention scale into transposed bf16 weights."""
    f = np.float32
    out = {}
    for i in (1, 2):
        g = np.asarray(inp[f"ln{i}_g"], f)
        out[f"wq{i}"] = (g[:, None] * np.asarray(inp[f"w_q{i}"], f).T
                         * np.float32(DH ** -0.5))
        out[f"wk{i}"] = g[:, None] * np.asarray(inp[f"w_k{i}"], f).T
        out[f"wv{i}"] = g[:, None] * np.asarray(inp[f"w_v{i}"], f).T
        out[f"wo{i}"] = np.asarray(inp[f"w_o{i}"], f).T
    g3 = np.asarray(inp["ln3_g"], f)
    wf1 = g3[:, None] * np.asarray(inp["w_ff1"], f).T      # [1024, 8192]
    # [kd, p, half, pm, col] -> [pm, p, (kd, half, col)]
    out["wf1"] = (wf1.reshape(8, P, 2, 32, P).transpose(3, 1, 0, 2, 4)
                  .reshape(32, P, 2048))
    out["wf2"] = np.asarray(inp["w_ff2"], f).T             # [4096, 1024]
    import ml_dtypes
    return {k: np.ascontiguousarray(v.astype(ml_dtypes.bfloat16))
            for k, v in out.items()}


def prep_biases(inp):
    """Bias vectors pushed through the projections (all-zero in practice)."""
    f = np.float32
    out = {}
    sc = np.float32(DH ** -0.5)
    for i in (1, 2):
        b = np.asarray(inp[f"ln{i}_b"], f)
        out[f"cq{i}"] = (np.asarray(inp[f"w_q{i}"], f) @ b * sc)[None, :]
        out[f"ck{i}"] = (np.asarray(inp[f"w_k{i}"], f) @ b)[None, :]
        out[f"cv{i}"] = (np.asarray(inp[f"w_v{i}"], f) @ b)[None, :]
        out[f"bo{i}"] = np.asarray(inp[f"b_o{i}"], f)[None, :]
    b3 = np.asarray(inp["ln3_b"], f)
    c1 = np.asarray(inp["w_ff1"], f) @ b3 + np.asarray(inp["b_ff1"], f)
    # reorder to the paired (a, gate) block layout used by wf1
    out["c1"] = c1.reshape(2, 32, P).transpose(1, 0, 2).reshape(1, 2 * FF)
    out["bf2"] = np.asarray(inp["b_ff2"], f)[None, :]
    import ml_dtypes
    return {k: np.ascontiguousarray(v.astype(ml_dtypes.bfloat16))
            for k, v in out.items()}


def any_bias(inp):
    keys = ["ln1_b", "ln2_b", "ln3_b", "b_o1", "b_o2", "b_ff1", "b_ff2"]
    return any(np.any(np.asarray(inp[k]) != 0) for k in keys)


# ======================================================================
# Host-side entry point: kernel(**inputs) -> full output [2, 2048, 1024]
# ======================================================================

_B, _N = 2, 2048
_NCORE = 8
_GROUP = 4
_TOK = _N // _GROUP

_cache = {}


def _get_nc(use_bias):
    key = ("nc", use_bias)
    if key not in _cache:
        _cache[key] = build(group=_GROUP, tok=_TOK, use_bias=use_bias)
    return _cache[key]


def kernel(**inputs):
    from concourse.bass_utils import run_bass_kernel_spmd

    inputs = {k: np.asarray(v) for k, v in inputs.items()}
    use_bias = any_bias(inputs)
    nc = _get_nc(use_bias)
    wdev = prep_weights(inputs)
    if use_bias:
        wdev.update(prep_biases(inputs))

    x = np.asarray(inputs["x"], np.float32)
    in_maps = []
    for core in range(_NCORE):
        b, p = core // _GROUP, core % _GROUP
        xs = np.ascontiguousarray(x[b, p * _TOK:(p + 1) * _TOK, :])
        in_maps.append({"x": xs, **wdev})

    res = run_bass_kernel_spmd(nc, in_maps, list(range(_NCORE)))

    y = np.zeros((_B, _N, D), np.float32)
    for core in range(_NCORE):
        b, p = core // _GROUP, core % _GROUP
        y[b, p * _TOK:(p + 1) * _TOK, :] = res.results[core]["y"]
    return y



# revision 5
# speedup vs baseline: 1.1234x; 1.1234x over previous
"""Trainium2 Bass kernel for nn_BasicTransformerBlock (self-contained).


Sharding: sequence-parallel. 8 cores = 2 batch groups x 4 cores.
Each core owns TOK=512 tokens of one batch element. Attention needs the
full 2048-token context of that batch element, obtained by AllGathering
K^T and V(+ones column) within the 4-core group.

Collective overlap: K/V are gathered in FOUR quarter-granularity
collectives (heads 4q..4q+3 each: 2 K^T feature tiles + the matching V
column block). Each quarter's projections are emitted, its gather is
kicked, and the attention inner loop consumes quarter q for head pairs
2q/2q+1 — so the ring runs concurrently with the projections and the
first half of the inner loop. A tiny warmup collective at kernel start
absorbs the one-time rank barrier (~37us).

Layout conventions per core:
  x residual stream: token-major [512, 1024] fp32, as 4 tiles [128, 1024].
  xlnT: LayerNorm'd x, transposed: [1024, 512] bf16 as 8 tiles [128, 512].
  Q^T, K^T: feature-major [1024, tok] bf16.
  V: token-major [tok, 1040] bf16 with a ones column appended per head
     (column h*65+64), so the AV matmul also produces the softmax
     denominator (row 64 of the [65, tok] output).
  kv bounce row layout (per partition): [K m=2q (512) | K m=2q+1 (512) |
     V t0..t3 (4x260)] = 2064 bf16 -> gathered [4 ranks, 128, 2064].
  Scores S^T: [j, q] per head in [128, 2, 512] psum tiles (2 j-tiles);
     one Exp per psum tile -> P^T bf16 in SBUF.
  attn out^T: feature-major [1024, tok] bf16 after per-head 1/denom scale.
"""


from contextlib import ExitStack

import numpy as np

import concourse.bass as bass
import concourse.mybir as mybir
import concourse.tile as tile
from concourse.tile_rust import add_dep_helper
from concourse import bacc
from concourse.masks import make_identity

F32 = mybir.dt.float32
BF16 = mybir.dt.bfloat16
AX = mybir.AxisListType.X
AF = mybir.ActivationFunctionType
ALU = mybir.AluOpType

D = 1024          # model dim
HEADS = 16
DH = 64
FF = 4096         # ff inner (per half)
EPS = 1e-5
P = 128
QW = 4 * 65       # V cols per quarter (4 heads x (64 + ones))

RG = [[0, 1, 2, 3], [4, 5, 6, 7]]


class Ctx:
    """Holds the bass handles shared across emit stages."""


def build(group: int, tok: int, use_bias: bool = False):
    """group: cores per batch group (1 = no collective, 4 = real).
    tok: local tokens per core (512)."""
    ntok = group * tok
    TT = tok // P          # local token tiles (4)
    JT = ntok // P         # context token tiles (16 when group=4)
    ROW = 2 * tok + 4 * QW  # kv bounce bytes(elems) per partition (2064)

    nc = bacc.Bacc("TRN2", target_bir_lowering=False, debug=False,
                   num_devices=8)

    c = Ctx()
    c.nc = nc
    c.group = group
    c.tok = tok
    c.ntok = ntok
    c.TT = TT
    c.JT = JT
    c.ROW = ROW
    c.use_bias = use_bias

    # ---- I/O ----
    c.x_in = nc.dram_tensor("x", [tok, D], F32, kind="ExternalInput")
    c.y_out = nc.dram_tensor("y", [tok, D], F32, kind="ExternalOutput")
    w = {}
    for i in (1, 2):
        w[f"wq{i}"] = nc.dram_tensor(f"wq{i}", [D, D], BF16, kind="ExternalInput")
        w[f"wk{i}"] = nc.dram_tensor(f"wk{i}", [D, D], BF16, kind="ExternalInput")
        w[f"wv{i}"] = nc.dram_tensor(f"wv{i}", [D, D], BF16, kind="ExternalInput")
        w[f"wo{i}"] = nc.dram_tensor(f"wo{i}", [D, D], BF16, kind="ExternalInput")
    w["wf1"] = nc.dram_tensor("wf1", [32, P, 2048], BF16, kind="ExternalInput")
    w["wf2"] = nc.dram_tensor("wf2", [FF, D], BF16, kind="ExternalInput")
    if use_bias:
        # per-projection bias vectors (LN beta pushed through W, plus the
        # projection's own bias where it exists)
        for i in (1, 2):
            for nmv in ("cq", "ck", "cv", "bo"):
                w[f"{nmv}{i}"] = nc.dram_tensor(f"{nmv}{i}", [1, D], BF16,
                                                kind="ExternalInput")
        w["c1"] = nc.dram_tensor("c1", [1, 2 * FF], BF16, kind="ExternalInput")
        w["bf2"] = nc.dram_tensor("bf2", [1, D], BF16, kind="ExternalInput")
    c.w = w

    with ExitStack() as stack:
        tc = stack.enter_context(tile.TileContext(nc))
        c.tc = tc

        const = stack.enter_context(tc.tile_pool(name="const", bufs=1))
        c.identity = const.tile([P, P], BF16, name="identity")
        make_identity(nc, c.identity)
        c.ones_bf64 = const.tile([1, 64], BF16, name="ones_bf64")
        nc.vector.memset(c.ones_bf64, 1.0)
        c.eps_tile = const.tile([P, 1], F32, name="eps_tile")
        nc.vector.memset(c.eps_tile, EPS)
        c.ones_bf = const.tile([1, tok], BF16, name="ones_bf")
        nc.vector.memset(c.ones_bf, 1.0)
        if use_bias:
            c.bias_sb = {}
            for key, t in w.items():
                if key[0] in "cb" and key not in ("c1",):
                    bt = const.tile([1, D], BF16, name=f"sb_{key}")
                    nc.sync.dma_start(bt, t[:, :])
                    c.bias_sb[key] = bt
            bt = const.tile([1, 2 * FF], BF16, name="sb_c1")
            nc.sync.dma_start(bt, w["c1"][:, :])
            c.bias_sb["c1"] = bt

        # warmup collective: absorbs the one-time rank barrier while the
        # input DMA + first LayerNorm run.
        if group > 1:
            warm_pool = stack.enter_context(
                tc.tile_pool(name="warm", bufs=1, space="DRAM"))
            warm_in = warm_pool.tile([64], BF16, name="warm_in")
            warm_out = warm_pool.tile([group * 64], BF16, name="warm_out")
            wsb = const.tile([1, 64], BF16, name="warm_sb")
            nc.vector.memset(wsb, 0.0)
            nc.sync.dma_start(warm_in.rearrange("(p f) -> p f", p=1), wsb)
            nc.gpsimd.collective_compute(
                "AllGather", ALU.bypass, replica_groups=RG,
                ins=[warm_in[:]], outs=[warm_out[:]])

        xres_pool = stack.enter_context(tc.tile_pool(name="xres", bufs=1))
        c.xres = [xres_pool.tile([P, D], F32, name=f"xres{t}") for t in range(TT)]
        for t in range(TT):
            nc.sync.dma_start(c.xres[t], c.x_in[t * P:(t + 1) * P, :])

        # DRAM bounce buffers: quarter-granularity combined K+V gathers
        if group > 1:
            dram = stack.enter_context(
                tc.tile_pool(name="dram", bufs=1, space="DRAM"))
            c.kv_in = [[dram.tile([P * ROW], BF16, name=f"kv_in{i}_{q}")
                        for q in range(4)] for i in (0, 1)]
            c.kv_out = [[dram.tile([group * P * ROW], BF16,
                                   name=f"kv_out{i}_{q}")
                         for q in range(4)] for i in (0, 1)]

        emit_attn(c, 1)
        emit_attn(c, 2)
        emit_ff(c)

        for t in range(TT):
            nc.sync.dma_start(c.y_out[t * P:(t + 1) * P, :], c.xres[t])

    nc.compile()
    return nc


def emit_ln_transpose(c, outer, name):
    """LayerNorm xres (gain/bias pre-folded into weights) and produce
    xlnT: 8 tiles [128, tok] bf16 (transposed normalized x).
    xlnT tiles live in `outer`; scratch pools are closed on return."""
    nc, tc = c.nc, c.tc
    TT = c.TT

    xlnT_pool = outer.enter_context(tc.tile_pool(name=f"{name}_xlnT", bufs=1))
    xlnT = [xlnT_pool.tile([P, c.tok], BF16, name=f"{name}_xlnT{d}")
            for d in range(8)]

    with ExitStack() as ph:
        pool = ph.enter_context(tc.tile_pool(name=f"{name}_ln", bufs=2))
        psum_sq = ph.enter_context(
            tc.tile_pool(name=f"{name}_psq", bufs=2, space="PSUM"))
        psum_tr = ph.enter_context(
            tc.tile_pool(name=f"{name}_ptr", bufs=4, space="PSUM"))

        for t in range(TT):
            xr = c.xres[t]
            # var = E[x^2] - mu^2: sum and sumsq run concurrently (DVE / ACT)
            ssum = pool.tile([P, 1], F32, tag="ssum", name=f"{name}_ssum{t}")
            nc.vector.reduce_sum(ssum, xr, axis=AX)
            sq_sink = psum_sq.tile([P, D], F32, tag="sq", name=f"{name}_sq{t}")
            sumsq = pool.tile([P, 1], F32, tag="sumsq", name=f"{name}_vs{t}")
            nc.scalar.activation(sq_sink, xr, AF.Square, accum_out=sumsq)
            mu = pool.tile([P, 1], F32, tag="mu", name=f"{name}_mu{t}")
            nc.vector.tensor_scalar_mul(mu, ssum, 1.0 / D)
            musq = pool.tile([P, 1], F32, tag="musq", name=f"{name}_msq{t}")
            nc.vector.tensor_mul(musq, mu, mu)
            bvar = pool.tile([P, 1], F32, tag="bvar", name=f"{name}_bv{t}")
            nc.vector.tensor_scalar(bvar, musq, -1.0, EPS,
                                    op0=ALU.mult, op1=ALU.add)
            std = pool.tile([P, 1], F32, tag="std", name=f"{name}_std{t}")
            nc.scalar.activation(std, sumsq, AF.Sqrt, bias=bvar,
                                 scale=1.0 / D)
            rstd = pool.tile([P, 1], F32, tag="rstd", name=f"{name}_rstd{t}")
            nc.vector.reciprocal(rstd, std)
            nmr = pool.tile([P, 1], F32, tag="nmr", name=f"{name}_nmr{t}")
            nc.vector.tensor_mul(nmr, mu, rstd)
            nc.vector.tensor_scalar_mul(nmr, nmr, -1.0)
            xln = pool.tile([P, D], BF16, tag="xln", name=f"{name}_xln{t}")
            nc.vector.tensor_scalar(xln, xr, rstd, nmr,
                                    op0=ALU.mult, op1=ALU.add)
            for dc in range(8):
                tp = psum_tr.tile([P, P], BF16, tag="tp",
                                  name=f"{name}_tp{t}_{dc}")
                nc.tensor.transpose(tp, xln[:, dc * P:(dc + 1) * P],
                                    c.identity)
                nc.vector.tensor_copy(xlnT[dc][:, t * P:(t + 1) * P], tp)
    return xlnT


def emit_attn(c, idx):
    nc, tc = c.nc, c.tc
    name = f"a{idx}"
    TT, JT, tok = c.TT, c.JT, c.tok
    NQ = 4                 # quarters (4 heads each)
    nrk = c.group if c.group > 1 else 1

    with ExitStack() as ph:
        # ---- weight loads first (gpsimd DGE queue; no data deps) ----
        wpool = ph.enter_context(tc.tile_pool(name=f"{name}_w", bufs=1))
        wt = {}
        for wn in ("wk", "wv", "wq", "wo"):
            for kd in range(8):
                t = wpool.tile([P, D], BF16, name=f"{name}_{wn}{kd}")
                nc.gpsimd.dma_start(t, c.w[f"{wn}{idx}"][kd * P:(kd + 1) * P, :])
                wt[(wn, kd)] = t

        xlnT = emit_ln_transpose(c, ph, name)

        kfull_pool = ph.enter_context(tc.tile_pool(name=f"{name}_kf", bufs=1))
        kT_m = [kfull_pool.tile([P, c.ntok], BF16, name=f"{name}_kTm{m}")
                for m in range(8)]
        vq = [[kfull_pool.tile([P, TT, QW], BF16, name=f"{name}_vq{q}_{r}")
               for r in range(nrk)] for q in range(NQ)]
        qT = [kfull_pool.tile([P, tok], BF16, name=f"{name}_qT{m}")
              for m in range(8)]

        stage_stack = ExitStack()
        kv_pool = stage_stack.enter_context(
            tc.tile_pool(name=f"{name}_kv", bufs=1))
        kT_loc = [kv_pool.tile([P, tok], BF16, name=f"{name}_kTl{m}")
                  for m in range(8)]
        vstage = [kv_pool.tile([P, 4 * QW], BF16, name=f"{name}_vst{t}")
                  for t in range(TT)]
        for t in range(TT):
            nc.vector.memset(
                vstage[t].rearrange("p (h e) -> p h e", e=65)[:, :, 64:65],
                1.0)

        # ---- per-quarter: K proj, V proj, gather kick, Q proj ----
        with ExitStack() as sub:
            psk = sub.enter_context(
                tc.tile_pool(name=f"{name}_psk", bufs=2, space="PSUM"))
            psv = sub.enter_context(
                tc.tile_pool(name=f"{name}_psv", bufs=2, space="PSUM"))
            psq = sub.enter_context(
                tc.tile_pool(name=f"{name}_psq2", bufs=2, space="PSUM"))

            for q in range(NQ):
                # K^T quarter: feature-major [128, tok] per m-tile
                for mm in range(2):
                    m = 2 * q + mm
                    ps = psk.tile([P, tok], F32, tag="k", name=f"{name}_kps{m}")
                    for kd in range(8):
                        nc.tensor.matmul(
                            ps, lhsT=wt[("wk", kd)][:, m * P:(m + 1) * P],
                            rhs=xlnT[kd], start=(kd == 0),
                            stop=(kd == 7 and not c.use_bias))
                    if c.use_bias:
                        nc.tensor.matmul(
                            ps, lhsT=c.bias_sb[f"ck{idx}"][0:1, m * P:(m + 1) * P],
                            rhs=c.ones_bf, start=False, stop=True)
                    nc.vector.tensor_copy(kT_loc[m], ps)
                # V quarter: token-major [128, 256] per t-tile
                for t in range(TT):
                    ps = psv.tile([P, 256], F32, tag="v", name=f"{name}_vps{q}_{t}")
                    for kd in range(8):
                        nc.tensor.matmul(
                            ps, lhsT=xlnT[kd][:, t * P:(t + 1) * P],
                            rhs=wt[("wv", kd)][:, q * 256:(q + 1) * 256],
                            start=(kd == 0),
                            stop=(kd == 7 and not c.use_bias))
                    if c.use_bias:
                        nc.tensor.matmul(
                            ps, lhsT=c.ones_bf[0:1, 0:P],
                            rhs=c.bias_sb[f"cv{idx}"][0:1, q * 256:(q + 1) * 256],
                            start=False, stop=True)
                    dst = vstage[t][:, q * QW:(q + 1) * QW].rearrange(
                        "p (h e) -> p h e", e=65)[:, :, 0:64]
                    nc.vector.tensor_copy(
                        dst, ps.rearrange("p (h e) -> p h e", e=64))

                if c.group > 1:
                    kv2d = c.kv_in[idx - 1][q].rearrange("(p x) -> p x", p=P)
                    for mm in range(2):
                        nc.sync.dma_start(kv2d[:, mm * tok:(mm + 1) * tok],
                                          kT_loc[2 * q + mm])
                    for t in range(TT):
                        nc.sync.dma_start(
                            kv2d[:, 2 * tok + t * QW:2 * tok + (t + 1) * QW],
                            vstage[t][:, q * QW:(q + 1) * QW])
                    nc.gpsimd.collective_compute(
                        "AllGather", ALU.bypass, replica_groups=RG,
                        ins=[c.kv_in[idx - 1][q][:]],
                        outs=[c.kv_out[idx - 1][q][:]])

                # Q^T quarter (overlaps the gather)
                for mm in range(2):
                    m = 2 * q + mm
                    ps = psq.tile([P, tok], F32, tag="q", name=f"{name}_qps{m}")
                    for kd in range(8):
                        nc.tensor.matmul(
                            ps, lhsT=wt[("wq", kd)][:, m * P:(m + 1) * P],
                            rhs=xlnT[kd], start=(kd == 0),
                            stop=(kd == 7 and not c.use_bias))
                    if c.use_bias:
                        nc.tensor.matmul(
                            ps, lhsT=c.bias_sb[f"cq{idx}"][0:1, m * P:(m + 1) * P],
                            rhs=c.ones_bf, start=False, stop=True)
                    nc.vector.tensor_copy(qT[m], ps)

        # ---- readbacks (sync queue; each waits on its gather) ----
        if c.group == 1:
            for m in range(8):
                nc.vector.tensor_copy(kT_m[m], kT_loc[m])
            for t in range(TT):
                for q in range(NQ):
                    nc.scalar.copy(vq[q][0][:, t, :],
                                   vstage[t][:, q * QW:(q + 1) * QW])
            stage_stack.close()
        else:
            stage_stack.close()
            for q in range(NQ):
                src = c.kv_out[idx - 1][q].rearrange(
                    "(r p x) -> p r x", r=c.group, p=P)
                for mm in range(2):
                    nc.sync.dma_start(
                        kT_m[2 * q + mm].rearrange("p (r f) -> p r f",
                                                   r=c.group),
                        src[:, :, mm * tok:(mm + 1) * tok])
                for r in range(c.group):
                    nc.sync.dma_start(
                        vq[q][r],
                        src[:, r, 2 * tok:].rearrange("p (t e) -> p t e",
                                                      t=TT))

        # ---- attention inner loop: head pairs, software-pipelined ----
        attnT_pool = ph.enter_context(tc.tile_pool(name=f"{name}_at", bufs=1))
        attnT = [attnT_pool.tile([P, tok], BF16, name=f"{name}_attnT{m}")
                 for m in range(8)]
        NR = JT // 2           # score rounds per head (2 j-tiles per round)

        with ExitStack() as sub:
            psum_sc = sub.enter_context(
                tc.tile_pool(name=f"{name}_psc", bufs=2, space="PSUM"))
            psum_av = sub.enter_context(
                tc.tile_pool(name=f"{name}_pav", bufs=3, space="PSUM"))
            psum_bc = sub.enter_context(
                tc.tile_pool(name=f"{name}_pbc", bufs=1, space="PSUM"))
            pT_pool = sub.enter_context(
                tc.tile_pool(name=f"{name}_pT", bufs=10))
            small = sub.enter_context(
                tc.tile_pool(name=f"{name}_small", bufs=2))

            av_tiles = {}

            def emit_av(entry):
                m2, pp, rr = entry
                for u in range(2):
                    jt = 2 * rr + u
                    r_, t_ = (jt // TT, jt % TT) if c.group > 1 else (0, jt % TT)
                    if c.group == 1:
                        r_ = 0
                    for s in range(2):
                        h = 2 * m2 + s
                        nc.tensor.matmul(
                            av_tiles[m2][s][0:65, :],
                            lhsT=vq[h // 4][r_][:, t_,
                                                (h % 4) * 65:(h % 4 + 1) * 65],
                            rhs=pp[s][:, u, :],
                            start=(jt == 0), stop=(jt == JT - 1))

            def emit_epilogue(m2):
                av_pair = av_tiles.pop(m2)
                for s in range(2):
                    po2 = s * 64
                    den1 = small.tile([1, tok], F32, tag="den",
                                      name=f"{name}_den{m2}_{s}")
                    nc.vector.tensor_copy(den1, av_pair[s][64:65, :])
                    rden = small.tile([1, tok], F32, tag="rden",
                                      name=f"{name}_rd{m2}_{s}")
                    nc.vector.reciprocal_approx_fast(rden, den1)
                    rden_bf = small.tile([1, tok], BF16, tag="rdenb",
                                         name=f"{name}_rdb{m2}_{s}")
                    nc.vector.tensor_copy(rden_bf, rden)
                    ps_b = psum_bc.tile([64, tok], F32, tag="bc",
                                        name=f"{name}_bc{m2}_{s}")
                    nc.tensor.matmul(ps_b, lhsT=c.ones_bf64[0:1, 0:64],
                                     rhs=rden_bf, start=True, stop=True)
                    rbc = small.tile([64, tok], F32, tag="rbc",
                                     name=f"{name}_rbc{m2}_{s}")
                    nc.vector.tensor_copy(rbc, ps_b)
                    nc.vector.tensor_tensor(attnT[m2][po2:po2 + 64, :],
                                            av_pair[s][0:64, :], rbc,
                                            op=ALU.mult)

            pend = []          # (m, p2, r) not yet AV-multiplied
            ep_ready = []      # pairs whose full AV chain has been emitted
            for m in range(8):       # head pair (2m, 2m+1)
                av_tiles[m] = [psum_av.tile([P, tok], F32, tag="av",
                                            name=f"{name}_av{m}_{s}")
                               for s in range(2)]
                for r in range(NR):
                    ps2 = [psum_sc.tile([P, 2, tok], F32, tag="sc",
                                        name=f"{name}_sc{m}_{r}_{s}")
                           for s in range(2)]
                    p2 = []
                    for s in range(2):
                        po = s * 64
                        for u in range(2):
                            jt = 2 * r + u
                            nc.tensor.matmul(
                                ps2[s][:, u, :],
                                lhsT=kT_m[m][po:po + 64,
                                             jt * P:(jt + 1) * P],
                                rhs=qT[m][po:po + 64, :],
                                start=True, stop=True)
                        p_sb = pT_pool.tile([P, 2, tok], BF16, tag="pT",
                                            name=f"{name}_p{m}_{r}_{s}")
                        nc.scalar.activation(p_sb, ps2[s], AF.Exp)
                        p2.append(p_sb)
                    if r == 2 and ep_ready:
                        emit_epilogue(ep_ready.pop(0))
                    pend.append((m, p2, r))
                    while len(pend) > 2:
                        ent = pend.pop(0)
                        emit_av(ent)
                        if ent[2] == NR - 1:
                            ep_ready.append(ent[0])
            while pend:
                ent = pend.pop(0)
                emit_av(ent)
                if ent[2] == NR - 1:
                    ep_ready.append(ent[0])
            while ep_ready:
                emit_epilogue(ep_ready.pop(0))

        # ---- out projection + residual, t-outer so LN-next can chase ----
        with ExitStack() as sub:
            psum_o = sub.enter_context(
                tc.tile_pool(name=f"{name}_po", bufs=4, space="PSUM"))
            for t in range(TT):
                ps_o = [psum_o.tile([P, 512], F32, tag="o",
                                    name=f"{name}_pso{t}_{n}")
                        for n in range(2)]
                for m in range(8):
                    for n in range(2):
                        nc.tensor.matmul(
                            ps_o[n],
                            lhsT=attnT[m][:, t * P:(t + 1) * P],
                            rhs=wt[("wo", m)][:, n * 512:(n + 1) * 512],
                            start=(m == 0),
                            stop=(m == 7 and not c.use_bias))
                if c.use_bias:
                    for n in range(2):
                        nc.tensor.matmul(
                            ps_o[n], lhsT=c.ones_bf[0:1, 0:P],
                            rhs=c.bias_sb[f"bo{idx}"][0:1,
                                                      n * 512:(n + 1) * 512],
                            start=False, stop=True)
                for n in range(2):
                    sl = slice(n * 512, (n + 1) * 512)
                    nc.vector.tensor_add(c.xres[t][:, sl], c.xres[t][:, sl],
                                         ps_o[n])


def emit_ff(c):
    nc, tc = c.nc, c.tc
    name = "ff"
    TT, tok = c.TT, c.tok

    with ExitStack() as ph:
        # wf2 fully resident: 8 batched DMAs of 4 row-tiles each, issued
        # up-front on the gpsimd queue so FF2 never stalls on weights.
        wf2_pool = ph.enter_context(tc.tile_pool(name=f"{name}_w2", bufs=1))
        w2b = []
        for g in range(8):
            t = wf2_pool.tile([P, 4, D], BF16, name=f"{name}_w2_{g}")
            nc.gpsimd.dma_start(
                t, c.w["wf2"][g * 512:(g + 1) * 512, :].rearrange(
                    "(a p) d -> p a d", p=P))
            w2b.append(t)

        xlnT = emit_ln_transpose(c, ph, name)

        h2_pool = ph.enter_context(tc.tile_pool(name=f"{name}_h2", bufs=1))
        h2T = [h2_pool.tile([P, tok], BF16, name=f"{name}_h2T{m}")
               for m in range(32)]

        with ExitStack() as sub:
            f1_pool = sub.enter_context(
                tc.tile_pool(name=f"{name}_f1", bufs=8))
            psum_ff = sub.enter_context(
                tc.tile_pool(name=f"{name}_pff", bufs=3, space="PSUM"))
            gl_pool = sub.enter_context(
                tc.tile_pool(name=f"{name}_gl", bufs=3))

            for pm in range(32):
                f1 = f1_pool.tile([P, 8, 256], BF16, tag="f1",
                                  name=f"{name}_f1_{pm}")
                nc.gpsimd.dma_start(
                    f1.rearrange("p a b -> p (a b)"), c.w["wf1"][pm, :, :])
                ps_a = psum_ff.tile([P, tok], F32, tag="ffa",
                                    name=f"{name}_fa{pm}")
                ps_g = psum_ff.tile([P, tok], F32, tag="ffg",
                                    name=f"{name}_fg{pm}")
                for kd in range(8):
                    nc.tensor.matmul(ps_a, lhsT=f1[:, kd, 0:128],
                                     rhs=xlnT[kd], start=(kd == 0),
                                     stop=(kd == 7 and not c.use_bias))
                for kd in range(8):
                    nc.tensor.matmul(ps_g, lhsT=f1[:, kd, 128:256],
                                     rhs=xlnT[kd], start=(kd == 0),
                                     stop=(kd == 7 and not c.use_bias))
                if c.use_bias:
                    nc.tensor.matmul(
                        ps_a, lhsT=c.bias_sb["c1"][0:1, pm * 256:pm * 256 + 128],
                        rhs=c.ones_bf, start=False, stop=True)
                    nc.tensor.matmul(
                        ps_g,
                        lhsT=c.bias_sb["c1"][0:1, pm * 256 + 128:pm * 256 + 256],
                        rhs=c.ones_bf, start=False, stop=True)
                gl = gl_pool.tile([P, tok], BF16, tag="gelu",
                                  name=f"{name}_gl{pm}")
                nc.scalar.activation(gl, ps_g, AF.Gelu)
                nc.vector.tensor_tensor(h2T[pm], ps_a, gl, op=ALU.mult)

        # FF2 + residual, t-outer so the final y DMAs can start early
        with ExitStack() as sub:
            psum_o = sub.enter_context(
                tc.tile_pool(name=f"{name}_po2", bufs=4, space="PSUM"))
            for t in range(TT):
                ps_o = [psum_o.tile([P, 512], F32, tag="o",
                                    name=f"{name}_pso{t}_{n}")
                        for n in range(2)]
                for m in range(32):
                    for n in range(2):
                        nc.tensor.matmul(
                            ps_o[n],
                            lhsT=h2T[m][:, t * P:(t + 1) * P],
                            rhs=w2b[m // 4][:, m % 4, n * 512:(n + 1) * 512],
                            start=(m == 0),
                            stop=(m == 31 and not c.use_bias))
                if c.use_bias:
                    for n in range(2):
                        nc.tensor.matmul(
                            ps_o[n], lhsT=c.ones_bf[0:1, 0:P],
                            rhs=c.bias_sb["bf2"][0:1, n * 512:(n + 1) * 512],
                            start=False, stop=True)
                for n in range(2):
                    sl = slice(n * 512, (n + 1) * 512)
                    nc.vector.tensor_add(c.xres[t][:, sl], c.xres[t][:, sl],
                                         ps_o[n])


# ---------------- host-side helpers ----------------

def prep_weights(inp):
    """Fold LN gains + attention scale into transposed bf16 weights."""
    f = np.float32
    out = {}
    for i in (1, 2):
        g = np.asarray(inp[f"ln{i}_g"], f)
        out[f"wq{i}"] = (g[:, None] * np.asarray(inp[f"w_q{i}"], f).T
                         * np.float32(DH ** -0.5))
        out[f"wk{i}"] = g[:, None] * np.asarray(inp[f"w_k{i}"], f).T
        out[f"wv{i}"] = g[:, None] * np.asarray(inp[f"w_v{i}"], f).T
        out[f"wo{i}"] = np.asarray(inp[f"w_o{i}"], f).T
    g3 = np.asarray(inp["ln3_g"], f)
    wf1 = g3[:, None] * np.asarray(inp["w_ff1"], f).T      # [1024, 8192]
    # [kd, p, half, pm, col] -> [pm, p, (kd, half, col)]
    out["wf1"] = (wf1.reshape(8, P, 2, 32, P).transpose(3, 1, 0, 2, 4)
                  .reshape(32, P, 2048))
    out["wf2"] = np.asarray(inp["w_ff2"], f).T             # [4096, 1024]
    import ml_dtypes
    return {k: np.ascontiguousarray(v.astype(ml_dtypes.bfloat16))
            for k, v in out.items()}


def prep_biases(inp):
    """Bias vectors pushed through the projections (all-zero in practice)."""
    f = np.float32
    out = {}
    sc = np.float32(DH ** -0.5)
    for i in (1, 2):
        b = np.asarray(inp[f"ln{i}_b"], f)
        out[f"cq{i}"] = (np.asarray(inp[f"w_q{i}"], f) @ b * sc)[None, :]
        out[f"ck{i}"] = (np.asarray(inp[f"w_k{i}"], f) @ b)[None, :]
        out[f"cv{i}"] = (np.asarray(inp[f"w_v{i}"], f) @ b)[None, :]
        out[f"bo{i}"] = np.asarray(inp[f"b_o{i}"], f)[None, :]
    b3 = np.asarray(inp["ln3_b"], f)
    c1 = np.asarray(inp["w_ff1"], f) @ b3 + np.asarray(inp["b_ff1"], f)
    # reorder to the paired (a, gate) block layout used by wf1
    out["c1"] = c1.reshape(2, 32, P).transpose(1, 0, 2).reshape(1, 2 * FF)
    out["bf2"] = np.asarray(inp["b_ff2"], f)[None, :]
    import ml_dtypes
    return {k: np.ascontiguousarray(v.astype(ml_dtypes.bfloat16))
            for k, v in out.items()}


def any_bias(inp):
    keys = ["ln1_b", "ln2_b", "ln3_b", "b_o1", "b_o2", "b_ff1", "b_ff2"]
    return any(np.any(np.asarray(inp[k]) != 0) for k in keys)


# ======================================================================
# Host-side entry point: kernel(**inputs) -> full output [2, 2048, 1024]
# ======================================================================

_B, _N = 2, 2048
_NCORE = 8
_GROUP = 4
_TOK = _N // _GROUP

_cache = {}


def _get_nc(use_bias):
    key = ("nc", use_bias)
    if key not in _cache:
        _cache[key] = build(group=_GROUP, tok=_TOK, use_bias=use_bias)
    return _cache[key]


def kernel(**inputs):
    from concourse.bass_utils import run_bass_kernel_spmd

    inputs = {k: np.asarray(v) for k, v in inputs.items()}
    use_bias = any_bias(inputs)
    nc = _get_nc(use_bias)
    wdev = prep_weights(inputs)
    if use_bias:
        wdev.update(prep_biases(inputs))

    x = np.asarray(inputs["x"], np.float32)
    in_maps = []
    for core in range(_NCORE):
        b, p = core // _GROUP, core % _GROUP
        xs = np.ascontiguousarray(x[b, p * _TOK:(p + 1) * _TOK, :])
        in_maps.append({"x": xs, **wdev})

    res = run_bass_kernel_spmd(nc, in_maps, list(range(_NCORE)))

    y = np.zeros((_B, _N, D), np.float32)
    for core in range(_NCORE):
        b, p = core // _GROUP, core % _GROUP
        y[b, p * _TOK:(p + 1) * _TOK, :] = res.results[core]["y"]
    return y
